# revision 1
# baseline (speedup 1.0000x reference)
"""Trainium2 Bass kernel for nn_Difference (ignorematch mode).

Math: result[i,j] = sum_k a_fk[i,k] * (a_fk[i,k] > 0) * (b_fk[j,k] <= 0)
where a_fk = a @ feats.T, b_fk = b @ feats.T.  This factorizes into three
matmuls with elementwise ops between them:

    P = relu(a @ feats.T)            # [Na, K]
    Q = (b @ feats.T) <= 0           # [Nb, K], exactly {0.0, 1.0}
    result = P @ Q.T                 # [Na, Nb]

No [Na, Nb, K] tensor is ever materialized.

Sharding: 2x4 grid over the output. Core (r, q) computes
result[r*512:(r+1)*512, q*256:(q+1)*256] from a-half r and b-quarter q;
feats is replicated. Inputs are pre-transposed AND pre-packed on host so
that (a) the contraction dim D lands on SBUF partitions (the PE reduces
over partitions) and (b) every DMA reads/writes one contiguous run per
partition (max DMA efficiency).

Precision: the b-side (mask) runs in fp32 — the sign of b_fk must match
the fp32 reference (min |b_fk| ~ 1e-4; reduced precision flips mask bits,
each costing an O(20-70) absolute error in the output). The a-side and
final matmul run in fp16 (1 cycle/row on the PE, half the DMA bytes;
measured absmax error ~0.4 out of |out|max ~2400).
"""

import os
import sys

import numpy as np

sys.path.insert(0, "/opt/trn_rl_repo")

import concourse.bacc as bacc  # noqa: E402
import concourse.tile as tile  # noqa: E402
from concourse import mybir  # noqa: E402
from concourse.bass_utils import run_bass_kernel_spmd  # noqa: E402

# Problem shapes (hardcoded per contract).
NA, NB, D, K = 1024, 1024, 512, 256
A_SPLIT, B_SPLIT = 2, 4  # 8 cores in a 2x4 grid over the output
IA = NA // A_SPLIT  # 512 output rows per core
JB = NB // B_SPLIT  # 256 output cols per core
P = 128
DC = D // P  # 4 contraction chunks
KC = K // P  # 2 feature-bank chunks
MC = IA // P  # 4 output row chunks
FB = K + JB  # packed feats+b row length per (partition, dc)

F32 = mybir.dt.float32
F16 = mybir.dt.float16

_BUILT = None
LAST_RESULTS = None


def _build():
    nc = bacc.Bacc("TRN2", target_bir_lowering=False, debug=False)

    # Packed inputs: one contiguous run per partition per DMA.
    # fb[p, dc, 0:K] = feats.T[dc*128+p, :], fb[p, dc, K:] = b.T[dc*128+p, jq]
    fb0 = nc.dram_tensor("fb0", [P, 2, FB], F32, kind="ExternalInput")  # dc 0,1
    fb1 = nc.dram_tensor("fb1", [P, 2, FB], F32, kind="ExternalInput")  # dc 2,3
    ah = nc.dram_tensor("ah", [P, DC, IA], F16, kind="ExternalInput")
    out = nc.dram_tensor("out", [P, MC, JB], F32, kind="ExternalOutput")

    with tile.TileContext(nc) as tc:
        with (
            tc.tile_pool(name="ins", bufs=1) as in_pool,
            tc.tile_pool(name="mid", bufs=1) as mid_pool,
            tc.tile_pool(name="outs", bufs=1) as out_pool,
            tc.tile_pool(name="ps_b", bufs=2, space="PSUM") as ps_b_pool,
            tc.tile_pool(name="ps_a", bufs=2, space="PSUM") as ps_a_pool,
            tc.tile_pool(name="ps_o", bufs=4, space="PSUM") as ps_o_pool,
        ):
            # PE warmup: the HAM clock gate keeps the PE at 1.2 GHz until it
            # has been busy ~3.4us. Run dummy matmuls on a zeroed tile while
            # the input DMAs are in flight so the real matmuls start at 2.4.
            warm_sb = in_pool.tile([P, 512], F16, tag="warm", name="warm_sb")
            nc.vector.memset(warm_sb[:], 0.0)
            ps_w = ps_b_pool.tile([P, 512], F32, tag="psb", name="ps_w")
            for _ in range(7):
                nc.tensor.matmul(
                    ps_w[:], lhsT=warm_sb[:, 0:P], rhs=warm_sb[:], start=True, stop=True
                )
            fb_sb = [
                in_pool.tile([P, 2, FB], F32, tag="fb0", name="fb_sb0"),
                in_pool.tile([P, 2, FB], F32, tag="fb1", name="fb_sb1"),
            ]
            ah_sb = in_pool.tile([P, DC, IA], F16, tag="ah")
            # One DMA per DGE path (SP-HWDGE, ACT-HWDGE, SWDGE): the ~2.4us
            # write-receipt before each completion-sem serializes per ring,
            # and total receipt count drives cross-core contention, so three
            # large parallel loads beat many small chunks.
            nc.sync.dma_start(out=fb_sb[0][:], in_=fb0[:])
            nc.scalar.dma_start(out=fb_sb[1][:], in_=fb1[:])
            nc.gpsimd.dma_start(out=ah_sb[:], in_=ah[:])

            def fT(dc):  # feats.T chunk [128d, 256k], f32
                return fb_sb[dc // 2][:, dc % 2, 0:K]

            def bT(dc):  # b.T chunk [128d, 256j], f32
                return fb_sb[dc // 2][:, dc % 2, K:FB]

            # a-side lhsT: feats cast to fp16 on-chip (b-side keeps raw f32)
            fh_sb = mid_pool.tile([P, DC, K], F16, tag="fh")
            for h in range(2):
                nc.vector.tensor_copy(
                    out=fh_sb[:, 2 * h : 2 * h + 2, :], in_=fb_sb[h][:, :, 0:K]
                )

            QT_sb = mid_pool.tile([P, KC, JB], F16, tag="qt")
            PT_sb = mid_pool.tile([P, KC, IA], F16, tag="pt")

            # b-side in fp32: QT[k, j] = 1.0 if b_fk[j, k] <= 0 else 0.0
            # dc-major order so MMs on the first fb chunk start while the
            # second chunk's DMA is still in flight.
            ps_b = [
                ps_b_pool.tile([P, JB], F32, tag="psb", name=f"ps_b{kc}")
                for kc in range(KC)
            ]
            for dc in range(DC):
                for kc in range(KC):
                    nc.tensor.matmul(
                        ps_b[kc][:],
                        lhsT=fT(dc)[:, kc * P : (kc + 1) * P],
                        rhs=bT(dc),
                        start=(dc == 0),
                        stop=(dc == DC - 1),
                    )
            for kc in range(KC):
                nc.vector.tensor_scalar(
                    QT_sb[:, kc, :], ps_b[kc][:], 0.0, None, mybir.AluOpType.is_le
                )

            # a-side in fp16 (PT[k, i] = relu(a_fk[i, k])), interleaved with
            # the final matmuls: right after the a-side kc-group finishes, the
            # final matmuls' kc partial products run while the next a-side
            # group streams — the relu eviction latency hides behind the PE.
            out_sb = out_pool.tile([P, MC, JB], F32, tag="osb")
            ps_o = [
                ps_o_pool.tile([P, JB], F32, tag="pso", name=f"ps_o{mc}")
                for mc in range(MC)
            ]
            for kc in range(KC):
                ps = ps_a_pool.tile([P, IA], F32, tag="psa")
                for dc in range(DC):
                    nc.tensor.matmul(
                        ps[:],
                        lhsT=fh_sb[:, dc, kc * P : (kc + 1) * P],
                        rhs=ah_sb[:, dc, :],
                        start=(dc == 0),
                        stop=(dc == DC - 1),
                    )
                # per-mc relu chunks alternating ACT/DVE: each final-matmul
                # m-chunk is unblocked by its own slice, two chunks in flight
                for mc in range(MC):
                    sl = slice(mc * P, (mc + 1) * P)
                    if mc % 2:
                        nc.vector.tensor_scalar_max(PT_sb[:, kc, sl], ps[:, sl], 0.0)
                    else:
                        nc.scalar.activation(
                            PT_sb[:, kc, sl],
                            ps[:, sl],
                            mybir.ActivationFunctionType.Relu,
                        )
                # final in fp16 (Q is exactly {0,1}): out[i,j] = sum_k PT*QT
                for mc in range(MC):
                    nc.tensor.matmul(
                        ps_o[mc][:],
                        lhsT=PT_sb[:, kc, mc * P : (mc + 1) * P],
                        rhs=QT_sb[:, kc, :],
                        start=(kc == 0),
                        stop=(kc == KC - 1),
                    )
                    if kc == KC - 1:
                        nc.vector.tensor_copy(out_sb[:, mc, :], ps_o[mc][:])
                        if mc % 2 == 1:
                            dma = nc.sync if mc == 1 else nc.scalar
                            dma.dma_start(
                                out=out[:, mc - 1 : mc + 1, :],
                                in_=out_sb[:, mc - 1 : mc + 1, :],
                            )

    nc.finalize()
    return nc


def kernel(a, b, feats):
    global _BUILT, LAST_RESULTS
    a = np.ascontiguousarray(a, dtype=np.float32)
    b = np.ascontiguousarray(b, dtype=np.float32)
    feats = np.ascontiguousarray(feats, dtype=np.float32)

    if _BUILT is None:
        _BUILT = _build()
    nc = _BUILT

    fT_full = np.ascontiguousarray(feats.T)  # [D, K]
    bT_full = np.ascontiguousarray(b.T)  # [D, NB]
    aT_h = a.T.astype(np.float16)  # [D, NA]

    # fb per (q): [P, DC, FB] with fb[:, dc, :K] = fT rows, fb[:, dc, K:] = bT rows
    fT_r = fT_full.reshape(DC, P, K)
    bT_r = bT_full.reshape(DC, P, NB)
    aT_r = aT_h.reshape(DC, P, NA)

    in_maps = []
    for r in range(A_SPLIT):
        for q in range(B_SPLIT):
            fb = np.empty((P, DC, FB), dtype=np.float32)
            fb[:, :, 0:K] = fT_r.transpose(1, 0, 2)
            fb[:, :, K:FB] = bT_r[:, :, q * JB : (q + 1) * JB].transpose(1, 0, 2)
            ah = aT_r[:, :, r * IA : (r + 1) * IA].transpose(1, 0, 2)
            in_maps.append(
                {
                    "fb0": np.ascontiguousarray(fb[:, 0:2, :]),
                    "fb1": np.ascontiguousarray(fb[:, 2:4, :]),
                    "ah": np.ascontiguousarray(ah),
                }
            )

    kwargs = {}
    if os.environ.get("KERNEL_TRACE"):
        try:
            import antenv.axon_hooks  # noqa: F401  (shimmed by test.py)

            kwargs = dict(trace=True, trace_cores=list(range(8)))
        except ImportError:
            pass
    res = run_bass_kernel_spmd(nc, in_maps, core_ids=list(range(8)), **kwargs)
    LAST_RESULTS = res

    out = np.empty((NA, NB), dtype=np.float32)
    for c, r_map in enumerate(res.results):
        r, q = divmod(c, B_SPLIT)
        # device out: [P, MC, JB]; rows of result tile are mc*128 + p
        tile_out = r_map["out"].transpose(1, 0, 2).reshape(IA, JB)
        out[r * IA : (r + 1) * IA, q * JB : (q + 1) * JB] = tile_out
    return out



# revision 4
# speedup vs baseline: 1.0096x; 1.0096x over previous
"""Trainium2 Bass kernel for nn_Difference (ignorematch mode).

Math: result[i,j] = sum_k a_fk[i,k] * (a_fk[i,k] > 0) * (b_fk[j,k] <= 0)
where a_fk = a @ feats.T, b_fk = b @ feats.T.  This factorizes into three
matmuls with elementwise ops between them:

    P = relu(a @ feats.T)            # [Na, K]
    Q = (b @ feats.T) <= 0           # [Nb, K], exactly {0.0, 1.0}
    result = P @ Q.T                 # [Na, Nb]

No [Na, Nb, K] tensor is ever materialized.

Sharding: 4x2 grid over the output. Core (r, q) computes
result[r*256:(r+1)*256, q*512:(q+1)*512] from a-quarter r and b-half q;
feats is replicated.  JB=512 makes the b-side and final matmuls full
512-column (one PSUM bank) passes, so LDWEIGHTS is always hidden.

Precision: everything runs in fp16.  The mask side (b_fk sign) flips on
|b_fk| < ~0.04: measured on the fixed test inputs this flips 18 of 262k
mask bits; together with fp16 a-side/output rounding the result's norm
rel err is 1.9e-3 vs the 2e-2 gate.  PE fp16 is 1 col/cycle vs fp32's
2 cycles/col/pass * 2 passes, and halves every DMA byte.

DMA plan (one DMA per DGE ring -- each completion pays a ~2.4us write
receipt that serializes per ring):
  Sync   HWDGE: fb01 = packed feats.T+b.T rows for d-chunks 0,1 (384KB)
  Scalar HWDGE: fb23 = same for d-chunks 2,3 (384KB)
  GpSimd SWDGE: ah   = a.T quarter, all 4 d-chunks (256KB, slow ring)
Outputs go out as two 128KB fp16 pieces on the two HWDGE rings so their
receipts overlap; host upcasts to f32 during the unshard.

PE warmup: the HAM clock gate keeps the PE at 1.2 GHz until it has been
busy ~4us.  Dummy matmuls on a zeroed tile run while the input DMAs are
in flight so the real matmuls start close to 2.4 GHz.
"""

import os
import sys

import numpy as np

sys.path.insert(0, "/opt/trn_rl_repo")

import concourse.bacc as bacc  # noqa: E402
import concourse.tile as tile  # noqa: E402
from concourse import mybir  # noqa: E402
from concourse.bass_utils import run_bass_kernel_spmd  # noqa: E402

# Problem shapes (hardcoded per contract).
NA, NB, D, K = 1024, 1024, 512, 256
A_SPLIT, B_SPLIT = 4, 2  # 8 cores in a 4x2 grid over the output
IA = NA // A_SPLIT  # 256 output rows per core
JB = NB // B_SPLIT  # 512 output cols per core
P = 128
DC = D // P  # 4 contraction chunks
KC = K // P  # 2 feature-bank chunks
MC = IA // P  # 2 output row chunks
FB = K + JB  # packed feats+b row length per (partition, dc): 768

F32 = mybir.dt.float32
F16 = mybir.dt.float16

N_WARM = 7  # dummy 512-col matmuls to ramp the PE clock during input DMA

_BUILT = None
LAST_RESULTS = None


def _build():
    nc = bacc.Bacc("TRN2", target_bir_lowering=False, debug=False)

    # Packed inputs: one contiguous run per partition per DMA.
    # fbH[p, h, 0:K] = feats.T[(2H+h)*128+p, :], fbH[p, h, K:] = b.T[.., jq]
    fb0 = nc.dram_tensor("fb0", [P, 2, FB], F16, kind="ExternalInput")  # dc 0,1
    fb1 = nc.dram_tensor("fb1", [P, 2, FB], F16, kind="ExternalInput")  # dc 2,3
    ah = nc.dram_tensor("ah", [P, DC, IA], F16, kind="ExternalInput")
    out = nc.dram_tensor("out", [P, MC, JB], F16, kind="ExternalOutput")

    with tile.TileContext(nc) as tc:
        with (
            tc.tile_pool(name="ins", bufs=1) as in_pool,
            tc.tile_pool(name="mid", bufs=1) as mid_pool,
            tc.tile_pool(name="outs", bufs=1) as out_pool,
            tc.tile_pool(name="ps_w", bufs=1, space="PSUM") as ps_w_pool,
            tc.tile_pool(name="ps_b", bufs=2, space="PSUM") as ps_b_pool,
            tc.tile_pool(name="ps_a", bufs=2, space="PSUM") as ps_a_pool,
            tc.tile_pool(name="ps_o", bufs=2, space="PSUM") as ps_o_pool,
        ):
            fb_sb = [
                in_pool.tile([P, 2, FB], F16, tag="fb0", name="fb_sb0"),
                in_pool.tile([P, 2, FB], F16, tag="fb1", name="fb_sb1"),
            ]
            ah_sb = in_pool.tile([P, DC, IA], F16, tag="ah")
            # One DMA per DGE ring, issued first on each issuing engine.
            nc.sync.dma_start(out=fb_sb[0][:], in_=fb0[:])
            nc.scalar.dma_start(out=fb_sb[1][:], in_=fb1[:])
            nc.gpsimd.dma_start(out=ah_sb[:], in_=ah[:])

            # PE clock warmup on a zeroed tile while the DMAs fly.
            warm_sb = in_pool.tile([P, 512], F16, tag="warm", name="warm_sb")
            nc.vector.memset(warm_sb[:], 0.0)
            ps_w = ps_w_pool.tile([P, 512], F32, tag="psw", name="ps_w")
            for _ in range(N_WARM):
                nc.tensor.matmul(
                    ps_w[:], lhsT=warm_sb[:, 0:P], rhs=warm_sb[:], start=True, stop=True
                )

            def fT(dc, kc):  # feats.T chunk [128d, 128k]
                return fb_sb[dc // 2][:, dc % 2, kc * P : (kc + 1) * P]

            def bT(dc):  # b.T chunk [128d, 512j]
                return fb_sb[dc // 2][:, dc % 2, K:FB]

            QT_sb = mid_pool.tile([P, KC, JB], F16, tag="qt")
            PT_sb = mid_pool.tile([P, KC, IA], F16, tag="pt")
            out_sb = out_pool.tile([P, MC, JB], F16, tag="osb")

            ps_b = [
                ps_b_pool.tile([P, JB], F32, tag="psb", name=f"ps_b{kc}")
                for kc in range(KC)
            ]
            ps_a = [
                ps_a_pool.tile([P, IA], F32, tag="psa", name=f"ps_a{kc}")
                for kc in range(KC)
            ]
            ps_o = [
                ps_o_pool.tile([P, JB], F32, tag="pso", name=f"ps_o{mc}")
                for mc in range(MC)
            ]

            # Main loop: per (kc, dc) one LDWEIGHTS feeds the b-side 512-col
            # and a-side 256-col passes.  kc-major so kc0's QT/PT are ready
            # while the PE streams kc1 -- the finals never stall.
            for kc in range(KC):
                for dc in range(DC):
                    nc.tensor.matmul(
                        ps_b[kc][:],
                        lhsT=fT(dc, kc),
                        rhs=bT(dc),
                        start=(dc == 0),
                        stop=(dc == DC - 1),
                    )
                    nc.tensor.matmul(
                        ps_a[kc][:],
                        lhsT=fT(dc, kc),
                        rhs=ah_sb[:, dc, :],
                        start=(dc == 0),
                        stop=(dc == DC - 1),
                    )

            # Elementwise: QT = (b_fk <= 0) on DVE, PT = relu(a_fk) on ACT
            # (GpSimd can't read PSUM).  Two engines run concurrently so
            # finals(kc) unblock ~one DVE op after the kc main loop retires.
            for kc in range(KC):
                nc.vector.tensor_scalar(
                    QT_sb[:, kc, :], ps_b[kc][:], 0.0, None, mybir.AluOpType.is_le
                )
                nc.scalar.activation(
                    PT_sb[:, kc, :], ps_a[kc][:], mybir.ActivationFunctionType.Relu
                )

            # Finals: out[i,j] = sum_k PT[k,i] * QT[k,j], 512-col passes.
            for kc in range(KC):
                for mc in range(MC):
                    nc.tensor.matmul(
                        ps_o[mc][:],
                        lhsT=PT_sb[:, kc, mc * P : (mc + 1) * P],
                        rhs=QT_sb[:, kc, :],
                        start=(kc == 0),
                        stop=(kc == KC - 1),
                    )

            # Evict + store: per-mc copy (cast f32->fp16) split across DVE
            # and ACT, then one 128KB DMA per HWDGE ring.
            nc.vector.tensor_copy(out_sb[:, 0, :], ps_o[0][:])
            nc.scalar.activation(
                out_sb[:, 1, :], ps_o[1][:], mybir.ActivationFunctionType.Copy
            )
            nc.sync.dma_start(out=out[:, 0, :], in_=out_sb[:, 0, :])
            nc.scalar.dma_start(out=out[:, 1, :], in_=out_sb[:, 1, :])

    nc.finalize()
    return nc


def kernel(a, b, feats):
    global _BUILT, LAST_RESULTS
    a = np.ascontiguousarray(a, dtype=np.float32)
    b = np.ascontiguousarray(b, dtype=np.float32)
    feats = np.ascontiguousarray(feats, dtype=np.float32)

    if _BUILT is None:
        _BUILT = _build()
    nc = _BUILT

    fT_r = np.ascontiguousarray(feats.T).astype(np.float16).reshape(DC, P, K)
    bT_r = np.ascontiguousarray(b.T).astype(np.float16).reshape(DC, P, NB)
    aT_r = np.ascontiguousarray(a.T).astype(np.float16).reshape(DC, P, NA)

    in_maps = []
    for r in range(A_SPLIT):
        for q in range(B_SPLIT):
            fb = np.empty((P, DC, FB), dtype=np.float16)
            fb[:, :, 0:K] = fT_r.transpose(1, 0, 2)
            fb[:, :, K:FB] = bT_r[:, :, q * JB : (q + 1) * JB].transpose(1, 0, 2)
            ah = aT_r[:, :, r * IA : (r + 1) * IA].transpose(1, 0, 2)
            in_maps.append(
                {
                    "fb0": np.ascontiguousarray(fb[:, 0:2, :]),
                    "fb1": np.ascontiguousarray(fb[:, 2:4, :]),
                    "ah": np.ascontiguousarray(ah),
                }
            )

    kwargs = {}
    if os.environ.get("KERNEL_TRACE"):
        try:
            import antenv.axon_hooks  # noqa: F401  (shimmed by test.py)

            kwargs = dict(trace=True, trace_cores=list(range(8)))
        except ImportError:
            pass
    res = run_bass_kernel_spmd(nc, in_maps, core_ids=list(range(8)), **kwargs)
    LAST_RESULTS = res

    out = np.empty((NA, NB), dtype=np.float32)
    for c, r_map in enumerate(res.results):
        r, q = divmod(c, B_SPLIT)
        # device out: [P, MC, JB]; rows of result tile are mc*128 + p
        tile_out = r_map["out"].transpose(1, 0, 2).reshape(IA, JB).astype(np.float32)
        out[r * IA : (r + 1) * IA, q * JB : (q + 1) * JB] = tile_out
    return out


# revision 6
# speedup vs baseline: 1.0949x; 1.0845x over previous
"""Trainium2 Bass kernel for nn_Difference (ignorematch mode).

Math: result[i,j] = sum_k a_fk[i,k] * (a_fk[i,k] > 0) * (b_fk[j,k] <= 0)
where a_fk = a @ feats.T, b_fk = b @ feats.T.  This factorizes into three
matmuls with elementwise ops between them:

    P = relu(a @ feats.T)            # [Na, K]
    Q = (b @ feats.T) <= 0           # [Nb, K], exactly {0.0, 1.0}
    result = P @ Q.T                 # [Na, Nb]

No [Na, Nb, K] tensor is ever materialized.

Sharding: 4x2 grid over the output. Core (r, q) computes
result[r*256:(r+1)*256, q*512:(q+1)*512] from a-quarter r and b-half q;
feats is replicated.  JB=512 makes the b-side and final matmuls full
512-column (one PSUM bank) passes, so LDWEIGHTS is always hidden.

Precision: everything runs in fp16.  The mask side (b_fk sign) flips on
|b_fk| < ~0.04: measured on the fixed test inputs this flips 18 of 262k
mask bits; together with fp16 a-side/output rounding the result's norm
rel err is 1.9e-3 vs the 2e-2 gate.  PE fp16 is 1 col/cycle vs fp32's
2 cycles/col/pass * 2 passes, and halves every DMA byte.

DMA plan (one DMA per DGE ring -- each completion pays a ~2.4us write
receipt that serializes per ring):
  Sync   HWDGE: fb01 = packed feats.T+b.T rows for d-chunks 0,1 (384KB)
  Scalar HWDGE: fb23 = same for d-chunks 2,3 (384KB)
  GpSimd SWDGE: ah   = a.T quarter, all 4 d-chunks (256KB, slow ring)
Outputs go out as two 128KB fp16 pieces on the two HWDGE rings so their
receipts overlap; host upcasts to f32 during the unshard.

PE warmup: the HAM clock gate keeps the PE at 1.2 GHz until it has been
busy ~4us.  Dummy matmuls on a zeroed tile run while the input DMAs are
in flight so the real matmuls start close to 2.4 GHz.
"""

import os
import sys

import numpy as np

sys.path.insert(0, "/opt/trn_rl_repo")

import concourse.bacc as bacc  # noqa: E402
import concourse.tile as tile  # noqa: E402
from concourse import mybir  # noqa: E402
from concourse.bass_utils import run_bass_kernel_spmd  # noqa: E402

# Problem shapes (hardcoded per contract).
NA, NB, D, K = 1024, 1024, 512, 256
A_SPLIT, B_SPLIT = 4, 2  # 8 cores in a 4x2 grid over the output
IA = NA // A_SPLIT  # 256 output rows per core
JB = NB // B_SPLIT  # 512 output cols per core
P = 128
DC = D // P  # 4 contraction chunks
KC = K // P  # 2 feature-bank chunks
MC = IA // P  # 2 output row chunks
FB = K + JB  # packed feats+b row length per (partition, dc): 768

F32 = mybir.dt.float32
F16 = mybir.dt.float16

N_WARM = 5  # dummy 512-col matmuls to ramp the PE clock during input DMA

_BUILT = None
LAST_RESULTS = None


def _build():
    nc = bacc.Bacc("TRN2", target_bir_lowering=False, debug=False)

    # Packed inputs: one contiguous run per partition per DMA.
    # fbH[p, h, 0:K] = feats.T[(2H+h)*128+p, :], fbH[p, h, K:] = b.T[.., jq]
    fb0 = nc.dram_tensor("fb0", [P, 2, FB], F16, kind="ExternalInput")  # dc 0,1
    fb1 = nc.dram_tensor("fb1", [P, 2, FB], F16, kind="ExternalInput")  # dc 2,3
    ah = nc.dram_tensor("ah", [P, DC, IA], F16, kind="ExternalInput")
    out = nc.dram_tensor("out", [P, MC, JB], F16, kind="ExternalOutput")

    with tile.TileContext(nc) as tc:
        with (
            tc.tile_pool(name="ins", bufs=1) as in_pool,
            tc.tile_pool(name="mid", bufs=1) as mid_pool,
            tc.tile_pool(name="outs", bufs=1) as out_pool,
            tc.tile_pool(name="ps_w", bufs=1, space="PSUM") as ps_w_pool,
            tc.tile_pool(name="ps_b", bufs=2, space="PSUM") as ps_b_pool,
            tc.tile_pool(name="ps_a", bufs=2, space="PSUM") as ps_a_pool,
            tc.tile_pool(name="ps_o", bufs=2, space="PSUM") as ps_o_pool,
        ):
            fb_sb = [
                in_pool.tile([P, 2, FB], F16, tag="fb0", name="fb_sb0"),
                in_pool.tile([P, 2, FB], F16, tag="fb1", name="fb_sb1"),
            ]
            ah_sb = in_pool.tile([P, DC, IA], F16, tag="ah")
            # One DMA per DGE ring, issued first on each issuing engine.
            nc.sync.dma_start(out=fb_sb[0][:], in_=fb0[:])
            nc.scalar.dma_start(out=fb_sb[1][:], in_=fb1[:])
            nc.gpsimd.dma_start(out=ah_sb[:], in_=ah[:])

            # PE clock warmup on a zeroed tile while the DMAs fly.
            warm_sb = in_pool.tile([P, 512], F16, tag="warm", name="warm_sb")
            nc.vector.memset(warm_sb[:], 0.0)
            ps_w = ps_w_pool.tile([P, 512], F32, tag="psw", name="ps_w")
            for _ in range(N_WARM):
                nc.tensor.matmul(
                    ps_w[:], lhsT=warm_sb[:, 0:P], rhs=warm_sb[:], start=True, stop=True
                )

            def fT(dc, kc):  # feats.T chunk [128d, 128k]
                return fb_sb[dc // 2][:, dc % 2, kc * P : (kc + 1) * P]

            def bT(dc):  # b.T chunk [128d, 512j]
                return fb_sb[dc // 2][:, dc % 2, K:FB]

            QT_sb = mid_pool.tile([P, KC, JB], F16, tag="qt")
            PT_sb = mid_pool.tile([P, KC, IA], F16, tag="pt")
            out_sb = out_pool.tile([P, MC, JB], F16, tag="osb")

            ps_b = [
                ps_b_pool.tile([P, JB], F32, tag="psb", name=f"ps_b{kc}")
                for kc in range(KC)
            ]
            ps_a = [
                ps_a_pool.tile([P, IA], F32, tag="psa", name=f"ps_a{kc}")
                for kc in range(KC)
            ]
            ps_o = [
                ps_o_pool.tile([P, JB], F32, tag="pso", name=f"ps_o{mc}")
                for mc in range(MC)
            ]

            # Main loop: ALL b-side passes first -- they only need fb on the
            # fast HWDGE rings, so the PE never stalls on the slow SWDGE ah
            # load (which lands ~1.5us later), and the PE stays gap-free
            # (idle gaps also appear to delay the HAM clock ramp-up).
            for kc in range(KC):
                for dc in range(DC):
                    nc.tensor.matmul(
                        ps_b[kc][:],
                        lhsT=fT(dc, kc),
                        rhs=bT(dc),
                        start=(dc == 0),
                        stop=(dc == DC - 1),
                    )
                # QT = (b_fk <= 0) on DVE right after its kc group; runs
                # under the next group's matmuls.
                nc.vector.tensor_scalar(
                    QT_sb[:, kc, :], ps_b[kc][:], 0.0, None, mybir.AluOpType.is_le
                )

            # a-side passes; relu on ACT right after each kc group (GpSimd
            # can't read PSUM), hidden under the next group / first finals.
            for kc in range(KC):
                for dc in range(DC):
                    nc.tensor.matmul(
                        ps_a[kc][:],
                        lhsT=fT(dc, kc),
                        rhs=ah_sb[:, dc, :],
                        start=(dc == 0),
                        stop=(dc == DC - 1),
                    )
                nc.scalar.activation(
                    PT_sb[:, kc, :], ps_a[kc][:], mybir.ActivationFunctionType.Relu
                )

            # Finals: out[i,j] = sum_k PT[k,i] * QT[k,j], 512-col passes.
            # kc0's deps (relu0) completed under the a-side kc1 group; kc1's
            # relu finishes under the kc0 finals.
            for kc in range(KC):
                for mc in range(MC):
                    nc.tensor.matmul(
                        ps_o[mc][:],
                        lhsT=PT_sb[:, kc, mc * P : (mc + 1) * P],
                        rhs=QT_sb[:, kc, :],
                        start=(kc == 0),
                        stop=(kc == KC - 1),
                    )

            # Evict + store: per-mc copy (cast f32->fp16) split across DVE
            # and ACT, then one 128KB DMA per HWDGE ring.
            nc.vector.tensor_copy(out_sb[:, 0, :], ps_o[0][:])
            nc.scalar.activation(
                out_sb[:, 1, :], ps_o[1][:], mybir.ActivationFunctionType.Copy
            )
            nc.sync.dma_start(out=out[:, 0, :], in_=out_sb[:, 0, :])
            nc.scalar.dma_start(out=out[:, 1, :], in_=out_sb[:, 1, :])

    nc.finalize()
    return nc


def kernel(a, b, feats):
    global _BUILT, LAST_RESULTS
    a = np.ascontiguousarray(a, dtype=np.float32)
    b = np.ascontiguousarray(b, dtype=np.float32)
    feats = np.ascontiguousarray(feats, dtype=np.float32)

    if _BUILT is None:
        _BUILT = _build()
    nc = _BUILT

    fT_r = np.ascontiguousarray(feats.T).astype(np.float16).reshape(DC, P, K)
    bT_r = np.ascontiguousarray(b.T).astype(np.float16).reshape(DC, P, NB)
    aT_r = np.ascontiguousarray(a.T).astype(np.float16).reshape(DC, P, NA)

    in_maps = []
    for r in range(A_SPLIT):
        for q in range(B_SPLIT):
            fb = np.empty((P, DC, FB), dtype=np.float16)
            fb[:, :, 0:K] = fT_r.transpose(1, 0, 2)
            fb[:, :, K:FB] = bT_r[:, :, q * JB : (q + 1) * JB].transpose(1, 0, 2)
            ah = aT_r[:, :, r * IA : (r + 1) * IA].transpose(1, 0, 2)
            in_maps.append(
                {
                    "fb0": np.ascontiguousarray(fb[:, 0:2, :]),
                    "fb1": np.ascontiguousarray(fb[:, 2:4, :]),
                    "ah": np.ascontiguousarray(ah),
                }
            )

    kwargs = {}
    if os.environ.get("KERNEL_TRACE"):
        try:
            import antenv.axon_hooks  # noqa: F401  (shimmed by test.py)

            kwargs = dict(trace=True, trace_cores=list(range(8)))
        except ImportError:
            pass
    res = run_bass_kernel_spmd(nc, in_maps, core_ids=list(range(8)), **kwargs)
    LAST_RESULTS = res

    out = np.empty((NA, NB), dtype=np.float32)
    for c, r_map in enumerate(res.results):
        r, q = divmod(c, B_SPLIT)
        # device out: [P, MC, JB]; rows of result tile are mc*128 + p
        tile_out = r_map["out"].transpose(1, 0, 2).reshape(IA, JB).astype(np.float32)
        out[r * IA : (r + 1) * IA, q * JB : (q + 1) * JB] = tile_out
    return out
